# revision 23
# baseline (speedup 1.0000x reference)
"""Trainium2 Bass kernel for nn_BCErrorCNN (dense_cnn), v3.

Network (per sample, input [17, 9]):
  Conv1D(128, k=3, relu) -> [15, 128]   (position 14 dead: never consumed)
  LocallyConnected1D(128, k=3, relu) -> [13, 128]  (position 12 dead)
  MaxPool1D(2) -> [6, 128]
  LocallyConnected1D(128, k=3, relu) -> [4, 128]
  GlobalAvgPool -> [128]; Dense(100, relu); Dense(1, sigmoid)

Sharding: pure data parallelism, batch 32768 -> 8 cores x 4096.

Fully fp16 datapath (PSUM accumulation fp32); measured rel err ~3e-4.
  - X transposed to [feature, batch] by the DMA XBAR straight out of DRAM:
    one [512,128]->[128,512] transpose per TA/TB per tile, both issued on
    the otherwise-idle SP (sync) engine so the ACT engine keeps its whole
    budget for PSUM evacuation.
  - conv reads TA/TB directly with zero-padded weights at legal 32-aligned
    base partitions (no strip DMAs); issue order alternates PE row bands
    so row-disjoint conv matmuls overlap in the array.
  - conv matmuls + lc1 triples interleaved in PE issue order so PSUM evac
    latency hides behind matmul work; conv PSUM singles with bufs=3.
  - lc1 evac fused with maxpool: ACT relu-evacs the even position, DVE
    scalar_tensor_tensor computes max(odd+bias, relu(even)) which equals
    relu(max(even+b, odd+b)) since relu(x) >= 0.
  - global-average-pool folded in front of Dense(100): S2 position sums
    on GPSIMD+DVE (wd1 pre-scaled by 1/4), so d1 is ONE matmul per tile.
  - d2 writes PSUM partition 96 (32-aligned PE column tile) so the tail
    needs only one [128,512] PSUM bank; per-tile sigmoid + output DMA.
"""

import functools

import numpy as np

# ---- constants (hardcoded per problem spec) --------------------------------
N_CORES = 8
B_FULL = 32768
BC = B_FULL // N_CORES  # per-core batch
NB = 512                # batch tile (columns per matmul)
NT = BC // NB           # batch tiles per core
LIN, CIN, F = 17, 9, 128
FEAT = LIN * CIN        # 153
NPOS = 14               # conv positions actually needed (0..13)
NL1 = 12                # lc1 positions needed (0..11)
NPOOL = 6
NL2 = 4
ND1 = 100

# Conv position p contracts feature rows 9p..9p+26.  TA holds features
# 0..127 on partitions 0..127, TB holds features 25..152.  The matmul
# base-partition rule constrains tile_position[0] by contraction size K:
# K<=32 -> {0,32,64,96}; K<=64 -> {0,64}; else 0.  q0 below is the
# partition where wc row 0 sits (TA: 9p; TB: 9p-25), base is the chosen
# 32-aligned start, K = q0 + 27 - base.
CONV_GEO = [
    (0, 0, 0), (0, 0, 9), (0, 0, 18), (0, 0, 27),      # p0..p3
    (0, 32, 36), (0, 0, 45), (0, 0, 54), (0, 0, 63),   # p4..p7
    (0, 64, 72), (0, 64, 81), (0, 64, 90), (0, 96, 99),  # p8..p11
    (1, 64, 83), (1, 64, 92),                          # p12, p13 (TB)
]
# Issue order: consecutive matmuls sit in disjoint PE row ranges wherever
# possible (only 4-5 and 6-7 conflict) so the systolic array overlaps them.
CONV_ORDER = [8, 0, 9, 1, 10, 2, 12, 3, 13, 4, 5, 11, 6, 7]


def _build_program(nt=NT, lc2_bias_zero=True):
    import concourse.tile as tile
    from concourse import bacc, mybir

    F32 = mybir.dt.float32
    F16 = mybir.dt.float16
    AF = mybir.ActivationFunctionType
    ALU = mybir.AluOpType

    bc = nt * NB
    nc = bacc.Bacc("TRN2", target_bir_lowering=False, debug=False,
                   num_devices=N_CORES)

    x = nc.dram_tensor("x", [bc * FEAT], F16, kind="ExternalInput").ap()
    wcp = nc.dram_tensor("wcp", [128, NPOS * F], F16, kind="ExternalInput").ap()
    w1 = nc.dram_tensor("w1", [128, NL1 * 3 * F], F16, kind="ExternalInput").ap()
    w2 = nc.dram_tensor("w2", [128, NL2 * 3 * F], F16, kind="ExternalInput").ap()
    wd1 = nc.dram_tensor("wd1", [F, ND1], F16, kind="ExternalInput").ap()
    wd2 = nc.dram_tensor("wd2", [ND1, 1], F16, kind="ExternalInput").ap()
    cb = nc.dram_tensor("cb", [F, 1], F32, kind="ExternalInput").ap()
    b1 = nc.dram_tensor("b1", [F, NL1], F32, kind="ExternalInput").ap()
    b2 = nc.dram_tensor("b2", [F, NL2], F32, kind="ExternalInput").ap()
    db = nc.dram_tensor("db", [ND1, 1], F32, kind="ExternalInput").ap()
    y = nc.dram_tensor("y", [bc], F32, kind="ExternalOutput").ap()

    with tile.TileContext(nc) as tc:
        with (
            tc.tile_pool(name="const", bufs=1) as cpool,
            tc.tile_pool(name="t", bufs=2) as tpool,
            tc.tile_pool(name="h", bufs=2) as hpool,
            tc.tile_pool(name="eo", bufs=3) as epool,
            tc.tile_pool(name="m", bufs=2) as mpool,
            tc.tile_pool(name="s2", bufs=2) as s2pool,
            tc.tile_pool(name="ss", bufs=2) as sspool,
            tc.tile_pool(name="s3", bufs=2) as s3pool,
            tc.tile_pool(name="ys", bufs=2) as ypool,
            tc.tile_pool(name="psC", bufs=2, space="PSUM") as psC,
            tc.tile_pool(name="psL", bufs=4, space="PSUM") as psL,
        ):
            def load_T(jt):
                # DMA-XBAR transpose: DRAM [batch, feat] -> SBUF [feat, batch]
                # One [512,128]->[128,512] instruction per target, on sync
                # (tile 0's TB goes on scalar so TA/TB land in parallel).
                TA = tpool.tile([128, NB], F16, tag="TA", name=f"TA{jt}")
                TB = tpool.tile([128, NB], F16, tag="TB", name=f"TB{jt}")
                b0 = jt * NB * FEAT
                srcA = x[b0:b0 + 1].copy()
                srcA.ap = srcA.ap[:0] + [[FEAT, NB], [1, 128]]
                nc.sync.dma_start(TA[:], srcA, transpose=True)
                srcB = x[b0 + 25:b0 + 26].copy()
                srcB.ap = srcB.ap[:0] + [[FEAT, NB], [1, 128]]
                nc.sync.dma_start(TB[:], srcB, transpose=True)
                return TA, TB

            T_pre = {0: load_T(0)}

            # ---- weights + biases: all on the gpsimd software DGE so the
            # hwdge (sync/scalar) DMA semaphores that conv/lc matmuls wait
            # on count only transposes, never multi-us weight transfers.
            # Order = first-use order: early conv weight blocks and the
            # biases the first evacuations read come before the bulk.
            wcpt = cpool.tile([128, NPOS * F], F16)
            nc.gpsimd.dma_start(wcpt[:, :11 * F], wcp[:, :11 * F])
            cbt = cpool.tile([F, 1], F32)
            nc.gpsimd.dma_start(cbt[:], cb[:])
            b1t = cpool.tile([F, NL1], F32)
            nc.gpsimd.dma_start(b1t[:], b1[:])
            nc.gpsimd.dma_start(wcpt[:, 11 * F:], wcp[:, 11 * F:])
            w1t = cpool.tile([128, NL1 * 3 * F], F16)
            nc.gpsimd.dma_start(w1t[:, :NL1 * 3 * F // 2],
                                w1[:, :NL1 * 3 * F // 2])
            nc.gpsimd.dma_start(w1t[:, NL1 * 3 * F // 2:],
                                w1[:, NL1 * 3 * F // 2:])
            w2t = cpool.tile([128, NL2 * 3 * F], F16)
            nc.gpsimd.dma_start(w2t[:], w2[:])
            wd1t = cpool.tile([F, ND1], F16)
            nc.gpsimd.dma_start(wd1t[:], wd1[:])
            wd2t = cpool.tile([ND1, 1], F16)
            nc.gpsimd.dma_start(wd2t[:], wd2[:])
            b2t = cpool.tile([F, NL2], F32)
            nc.gpsimd.dma_start(b2t[:], b2[:])
            dbt = cpool.tile([ND1, 1], F32)
            nc.gpsimd.dma_start(dbt[:], db[:])

            state = {}

            def conv_pair(a, TA, TB, H, it):
                # two row-band-disjoint positions share one [128,1024] psum
                # tile and a single strided evacuation into H.  PSUM half 0
                # always holds the lower position so the H stride stays
                # positive regardless of issue order.
                pi, pj = CONV_ORDER[2 * a], CONV_ORDER[2 * a + 1]
                lo, hi = min(pi, pj), max(pi, pj)
                ps = psC.tile([128, 2 * NB], F32, tag="C", name=f"pC{it}_{a}")
                for p in (pi, pj):
                    src, base, q0 = CONV_GEO[p]
                    K = q0 + 27 - base
                    T = TA if src == 0 else TB
                    half = 0 if p == lo else 1
                    nc.tensor.matmul(
                        ps[:, half * NB:(half + 1) * NB],
                        wcpt[base:base + K, p * F:(p + 1) * F],
                        T[base:base + K, :],
                        start=True, stop=True, tile_position=(base, 0))
                hdst = H[:, lo * NB:lo * NB + 1].copy()
                hdst.ap = hdst.ap[:1] + [[(hi - lo) * NB, 2], [1, NB]]
                if a % 2 == 0:
                    nc.scalar.activation(hdst, ps[:], AF.Relu, bias=cbt[:])
                else:
                    nc.vector.tensor_scalar(hdst, ps[:], cbt[:], 0.0,
                                            op0=ALU.add, op1=ALU.max)

            def lc1_triple(l, H, M, it):
                ps = psL.tile([128, NB], F32, tag="L", name=f"pL{it}_{l}")
                for k in range(3):
                    nc.tensor.matmul(
                        ps[:], w1t[:, (l * 3 + k) * F:(l * 3 + k + 1) * F],
                        H[:, (l + k) * NB:(l + k + 1) * NB],
                        start=(k == 0), stop=(k == 2))
                t = l // 2
                if l % 2 == 0:
                    EO = epool.tile([128, NB], F16, tag="E",
                                    name=f"E{it}_{t}")
                    nc.scalar.activation(EO[:], ps[:], AF.Relu,
                                         bias=b1t[:, l:l + 1])
                    state[("eo", t)] = EO
                else:
                    EO = state.pop(("eo", t))
                    nc.vector.scalar_tensor_tensor(
                        M[:, t * NB:(t + 1) * NB], ps[:], b1t[:, l:l + 1],
                        EO[:], op0=ALU.add, op1=ALU.max)

            def lc2_mean(M, S2, it, last=False):
                # lc2 + global-average fold.  Zero-bias path fuses the mean
                # into the evacuations: A_l = relu(ps_l) for l=0,1 (ACT),
                # then DVE scalar_tensor_tensor accumulates relu(ps_{l+2})
                # on top, and one DVE add produces Ssum.
                pss = []
                for l in range(NL2):
                    ps = psL.tile([128, NB], F32, tag="L", name=f"pT{it}_{l}")
                    for k in range(3):
                        nc.tensor.matmul(
                            ps[:], w2t[:, (l * 3 + k) * F:(l * 3 + k + 1) * F],
                            M[:, (l + k) * NB:(l + k + 1) * NB],
                            start=(k == 0), stop=(k == 2))
                    pss.append(ps)
                Ssum = sspool.tile([128, NB], F16, tag="S", name=f"Ss{it}")
                if lc2_bias_zero:
                    A = sspool.tile([128, 2 * NB], F16, tag="A", name=f"A{it}")
                    for h in range(2):
                        nc.scalar.activation(A[:, h * NB:(h + 1) * NB],
                                             pss[h][:], AF.Relu)
                        nc.vector.scalar_tensor_tensor(
                            S2[:, h * NB:(h + 1) * NB], pss[h + 2][:], 0.0,
                            A[:, h * NB:(h + 1) * NB],
                            op0=ALU.max, op1=ALU.add)
                    if last:
                        state[("s2half", it)] = S2
                        return
                    nc.vector.tensor_tensor(Ssum[:], S2[:, 0:NB], S2[:, NB:2 * NB],
                                            op=ALU.add)
                else:
                    for l in range(NL2):
                        sdst = S2[:, l * NB:(l + 1) * NB]
                        if l % 2 == 0:
                            nc.scalar.activation(sdst, pss[l][:], AF.Relu,
                                                 bias=b2t[:, l:l + 1])
                        else:
                            nc.vector.tensor_scalar(
                                sdst, pss[l][:], b2t[:, l:l + 1], 0.0,
                                op0=ALU.add, op1=ALU.max)
                    A = sspool.tile([128, 2 * NB], F16, tag="A", name=f"A{it}")
                    nc.gpsimd.tensor_tensor(A[:, 0:NB], S2[:, 0:NB],
                                            S2[:, NB:2 * NB], op=ALU.add)
                    nc.gpsimd.tensor_tensor(A[:, NB:2 * NB],
                                            S2[:, 2 * NB:3 * NB],
                                            S2[:, 3 * NB:4 * NB], op=ALU.add)
                    if last:
                        state[("s2half", it)] = A
                        return
                    nc.vector.tensor_tensor(Ssum[:], A[:, 0:NB],
                                            A[:, NB:2 * NB], op=ALU.add)
                state[("ss", it)] = Ssum

            def tail_a(it, last=False):
                # d1 matmul (mean already folded: wd1 pre-scaled by 1/4).
                # For the final tile there is no next-tile conv to hide the
                # mean chain behind, so accumulate the four S2 blocks on the
                # PE instead (S2 evacs are ready much earlier than Ssum).
                pD = psL.tile([128, NB], F32, tag="L", name=f"pD{it}")
                if last:
                    Shalf = state.pop(("s2half", it))
                    for h in range(2):
                        nc.tensor.matmul(pD[0:ND1, :], wd1t[:],
                                         Shalf[:, h * NB:(h + 1) * NB],
                                         start=(h == 0), stop=(h == 1))
                else:
                    Ssum = state.pop(("ss", it))
                    nc.tensor.matmul(pD[0:ND1, :], wd1t[:], Ssum[:],
                                     start=True, stop=True)
                state[("pd", it)] = pD

            def tail_b(it):
                pD = state.pop(("pd", it))
                S3 = s3pool.tile([ND1, NB], F16, tag="S3", name=f"S3{it}")
                nc.scalar.activation(S3[:], pD[0:ND1, :], AF.Relu,
                                     bias=dbt[:])
                # d2 reuses row 0 of the same bank; the WAR on d1's region is
                # already ordered by the S3 evac that d2 depends on.
                nc.tensor.matmul(pD[0:1, :], wd2t[:], S3[:],
                                 start=True, stop=True)
                Ys = ypool.tile([1, NB], F32, tag="Y", name=f"Y{it}")
                nc.scalar.activation(Ys[:], pD[0:1, :], AF.Sigmoid)
                nc.sync.dma_start(y[it * NB:(it + 1) * NB], Ys[0:1, :])

            for it in range(nt):
                TA, TB = T_pre.pop(it)
                if it + 1 < nt:
                    T_pre[it + 1] = load_T(it + 1)
                H = hpool.tile([128, NPOS * NB], F16, tag="H", name=f"H{it}")
                M = mpool.tile([128, NPOOL * NB], F16, tag="M", name=f"M{it}")
                S2 = s2pool.tile([128, NL2 * NB], F16, tag="S2",
                                 name=f"S2{it}")

                # conv pairs + lc1 triples interleaved; the previous tile's
                # serial tail chain (mean->d1->relu->d2) hides behind convs.
                conv_pair(0, TA, TB, H, it)    # p8, p0
                conv_pair(1, TA, TB, H, it)    # p9, p1
                conv_pair(2, TA, TB, H, it)    # p10, p2
                conv_pair(3, TA, TB, H, it)    # p12, p3
                lc1_triple(0, H, M, it)
                if it > 0:
                    tail_a(it - 1)
                lc1_triple(8, H, M, it)
                if it > 0:
                    tail_b(it - 1)
                conv_pair(4, TA, TB, H, it)    # p13, p4
                lc1_triple(1, H, M, it)
                conv_pair(5, TA, TB, H, it)    # p5, p11
                lc1_triple(2, H, M, it)
                conv_pair(6, TA, TB, H, it)    # p6, p7
                for l in (9, 10, 11, 3, 4, 5, 6, 7):
                    lc1_triple(l, H, M, it)
                lc2_mean(M, S2, it, last=(it == nt - 1))

            tail_a(nt - 1, last=True)
            tail_b(nt - 1)

    nc.compile()
    return nc


@functools.lru_cache(maxsize=4)
def _get_program(nt, lc2_bias_zero=True):
    return _build_program(nt, lc2_bias_zero)


def _prep_in_maps(inputs, conv_w, conv_b, lc1_w, lc1_b, lc2_w, lc2_b,
                  d1_w, d1_b, d2_w, nt=NT, n_cores=N_CORES):
    bc = nt * NB
    lc2_bias_zero = not np.any(lc2_b)
    f16, f32 = np.float16, np.float32

    wc = np.asarray(conv_w, dtype=f32).reshape(27, F)
    wcp_np = np.zeros((128, NPOS * F), dtype=f16)
    for p, (_, _, q0) in enumerate(CONV_GEO):
        wcp_np[q0:q0 + 27, p * F:(p + 1) * F] = wc
    w1_np = np.ascontiguousarray(
        np.asarray(lc1_w[:NL1], dtype=f32).reshape(NL1, 3, F, F)
        .transpose(2, 0, 1, 3).reshape(128, NL1 * 3 * F)).astype(f16)
    w2_np = np.ascontiguousarray(
        np.asarray(lc2_w, dtype=f32).reshape(NL2, 3, F, F)
        .transpose(2, 0, 1, 3).reshape(128, NL2 * 3 * F)).astype(f16)
    wd1_np = (np.asarray(d1_w, dtype=f32) * 0.25).astype(f16)
    wd2_np = np.asarray(d2_w, dtype=f32).reshape(ND1, 1).astype(f16)
    cb_np = np.ascontiguousarray(conv_b.reshape(F, 1), dtype=f32)
    b1_np = np.ascontiguousarray(np.asarray(lc1_b[:NL1], dtype=f32).T)
    b2_np = np.ascontiguousarray(np.asarray(lc2_b, dtype=f32).T)
    db_np = np.ascontiguousarray(d1_b.reshape(ND1, 1), dtype=f32)
    shared = dict(wcp=wcp_np, w1=w1_np, w2=w2_np, wd1=wd1_np, wd2=wd2_np,
                  cb=cb_np, b1=b1_np, b2=b2_np, db=db_np)

    x16 = np.asarray(inputs, dtype=f16).reshape(n_cores, bc * FEAT)
    in_maps = [dict(shared, x=x16[c]) for c in range(n_cores)]
    return in_maps, lc2_bias_zero


def kernel(inputs, conv_w, conv_b, lc1_w, lc1_b, lc2_w, lc2_b,
           d1_w, d1_b, d2_w):
    from concourse.bass_utils import run_bass_kernel_spmd

    in_maps, lc2_bias_zero = _prep_in_maps(
        inputs, conv_w, conv_b, lc1_w, lc1_b, lc2_w, lc2_b, d1_w, d1_b, d2_w)
    nc = _get_program(NT, lc2_bias_zero)
    res = run_bass_kernel_spmd(nc, in_maps, list(range(N_CORES)))
    out = np.concatenate([res.results[c]["y"] for c in range(N_CORES)])
    return out.reshape(B_FULL, 1).astype(np.float32)


# revision 26
# speedup vs baseline: 1.0328x; 1.0328x over previous
"""Trainium2 Bass kernel for nn_BCErrorCNN (dense_cnn), v3.

Network (per sample, input [17, 9]):
  Conv1D(128, k=3, relu) -> [15, 128]   (position 14 dead: never consumed)
  LocallyConnected1D(128, k=3, relu) -> [13, 128]  (position 12 dead)
  MaxPool1D(2) -> [6, 128]
  LocallyConnected1D(128, k=3, relu) -> [4, 128]
  GlobalAvgPool -> [128]; Dense(100, relu); Dense(1, sigmoid)

Sharding: pure data parallelism, batch 32768 -> 8 cores x 4096.

Fully fp16 datapath (PSUM accumulation fp32); measured rel err ~3e-4.
  - X transposed to [feature, batch] by the DMA XBAR straight out of DRAM:
    one [512,128]->[128,512] transpose per TA/TB per tile, both issued on
    the otherwise-idle SP (sync) engine so the ACT engine keeps its whole
    budget for PSUM evacuation.
  - conv reads TA/TB directly with zero-padded weights at legal 32-aligned
    base partitions (no strip DMAs); issue order alternates PE row bands
    so row-disjoint conv matmuls overlap in the array.
  - conv matmuls + lc1 triples interleaved in PE issue order so PSUM evac
    latency hides behind matmul work; conv PSUM singles with bufs=3.
  - lc1 evac fused with maxpool: ACT relu-evacs the even position, DVE
    scalar_tensor_tensor computes max(odd+bias, relu(even)) which equals
    relu(max(even+b, odd+b)) since relu(x) >= 0.
  - global-average-pool folded in front of Dense(100): S2 position sums
    on GPSIMD+DVE (wd1 pre-scaled by 1/4), so d1 is ONE matmul per tile.
  - d2 writes PSUM partition 96 (32-aligned PE column tile) so the tail
    needs only one [128,512] PSUM bank; per-tile sigmoid + output DMA.
"""

import functools

import numpy as np

# ---- constants (hardcoded per problem spec) --------------------------------
N_CORES = 8
B_FULL = 32768
BC = B_FULL // N_CORES  # per-core batch
NB = 512                # batch tile (columns per matmul)
NT = BC // NB           # batch tiles per core
LIN, CIN, F = 17, 9, 128
FEAT = LIN * CIN        # 153
NPOS = 14               # conv positions actually needed (0..13)
NL1 = 12                # lc1 positions needed (0..11)
NPOOL = 6
NL2 = 4
ND1 = 100

# Conv position p contracts feature rows 9p..9p+26.  TA holds features
# 0..127 on partitions 0..127, TB holds features 25..152.  The matmul
# base-partition rule constrains tile_position[0] by contraction size K:
# K<=32 -> {0,32,64,96}; K<=64 -> {0,64}; else 0.  q0 below is the
# partition where wc row 0 sits (TA: 9p; TB: 9p-25), base is the chosen
# 32-aligned start, K = q0 + 27 - base.
CONV_GEO = [
    (0, 0, 0), (0, 0, 9), (0, 0, 18), (0, 0, 27),      # p0..p3
    (0, 32, 36), (0, 0, 45), (0, 0, 54), (0, 0, 63),   # p4..p7
    (0, 64, 72), (0, 64, 81), (0, 64, 90), (0, 96, 99),  # p8..p11
    (1, 64, 83), (1, 64, 92),                          # p12, p13 (TB)
]
# Issue order: consecutive matmuls sit in disjoint PE row ranges where
# possible so the systolic array overlaps them.
CONV_ORDER = [0, 8, 1, 9, 2, 10, 5, 11, 4, 12, 3, 13, 6, 7]


def _build_program(nt=NT, lc2_bias_zero=True):
    import concourse.tile as tile
    from concourse import bacc, mybir

    F32 = mybir.dt.float32
    F16 = mybir.dt.float16
    AF = mybir.ActivationFunctionType
    ALU = mybir.AluOpType

    bc = nt * NB
    nc = bacc.Bacc("TRN2", target_bir_lowering=False, debug=False,
                   num_devices=N_CORES)

    x = nc.dram_tensor("x", [bc * FEAT], F16, kind="ExternalInput").ap()
    wcp = nc.dram_tensor("wcp", [128, NPOS * F], F16, kind="ExternalInput").ap()
    w1 = nc.dram_tensor("w1", [128, NL1 * 3 * F], F16, kind="ExternalInput").ap()
    w2 = nc.dram_tensor("w2", [128, NL2 * 3 * F], F16, kind="ExternalInput").ap()
    wd1 = nc.dram_tensor("wd1", [F, ND1], F16, kind="ExternalInput").ap()
    wd2 = nc.dram_tensor("wd2", [ND1, 1], F16, kind="ExternalInput").ap()
    cb = nc.dram_tensor("cb", [F, 1], F32, kind="ExternalInput").ap()
    b1 = nc.dram_tensor("b1", [F, NL1], F32, kind="ExternalInput").ap()
    b2 = nc.dram_tensor("b2", [F, NL2], F32, kind="ExternalInput").ap()
    db = nc.dram_tensor("db", [ND1, 1], F32, kind="ExternalInput").ap()
    y = nc.dram_tensor("y", [bc], F32, kind="ExternalOutput").ap()

    with tile.TileContext(nc) as tc:
        with (
            tc.tile_pool(name="const", bufs=1) as cpool,
            tc.tile_pool(name="t", bufs=2) as tpool,
            tc.tile_pool(name="h", bufs=2) as hpool,
            tc.tile_pool(name="eo", bufs=3) as epool,
            tc.tile_pool(name="m", bufs=2) as mpool,
            tc.tile_pool(name="s2", bufs=2) as s2pool,
            tc.tile_pool(name="ss", bufs=2) as sspool,
            tc.tile_pool(name="s3", bufs=2) as s3pool,
            tc.tile_pool(name="ys", bufs=2) as ypool,
            tc.tile_pool(name="psC", bufs=2, space="PSUM") as psC,
            tc.tile_pool(name="psL", bufs=4, space="PSUM") as psL,
        ):
            def load_T(jt):
                # DMA-XBAR transpose: DRAM [batch, feat] -> SBUF [feat, batch]
                # One [512,128]->[128,512] instruction per target, on sync
                # (tile 0's TB goes on scalar so TA/TB land in parallel).
                TA = tpool.tile([128, NB], F16, tag="TA", name=f"TA{jt}")
                TB = tpool.tile([128, NB], F16, tag="TB", name=f"TB{jt}")
                b0 = jt * NB * FEAT
                srcA = x[b0:b0 + 1].copy()
                srcA.ap = srcA.ap[:0] + [[FEAT, NB], [1, 128]]
                nc.sync.dma_start(TA[:], srcA, transpose=True)
                srcB = x[b0 + 25:b0 + 26].copy()
                srcB.ap = srcB.ap[:0] + [[FEAT, NB], [1, 128]]
                eng = nc.scalar if jt == 0 else nc.sync
                eng.dma_start(TB[:], srcB, transpose=True)
                return TA, TB

            T_pre = {0: load_T(0)}

            # ---- weights + biases: all on the gpsimd software DGE so the
            # hwdge (sync/scalar) DMA semaphores that conv/lc matmuls wait
            # on count only transposes, never multi-us weight transfers.
            # Order = first-use order: early conv weight blocks and the
            # biases the first evacuations read come before the bulk.
            wcpt = cpool.tile([128, NPOS * F], F16)
            nc.gpsimd.dma_start(wcpt[:, :11 * F], wcp[:, :11 * F])
            cbt = cpool.tile([F, 1], F32)
            nc.gpsimd.dma_start(cbt[:], cb[:])
            b1t = cpool.tile([F, NL1], F32)
            nc.gpsimd.dma_start(b1t[:], b1[:])
            nc.gpsimd.dma_start(wcpt[:, 11 * F:], wcp[:, 11 * F:])
            w1t = cpool.tile([128, NL1 * 3 * F], F16)
            nc.gpsimd.dma_start(w1t[:, :NL1 * 3 * F // 2],
                                w1[:, :NL1 * 3 * F // 2])
            nc.gpsimd.dma_start(w1t[:, NL1 * 3 * F // 2:],
                                w1[:, NL1 * 3 * F // 2:])
            w2t = cpool.tile([128, NL2 * 3 * F], F16)
            nc.gpsimd.dma_start(w2t[:], w2[:])
            wd1t = cpool.tile([F, ND1], F16)
            nc.gpsimd.dma_start(wd1t[:], wd1[:])
            wd2t = cpool.tile([ND1, 1], F16)
            nc.gpsimd.dma_start(wd2t[:], wd2[:])
            b2t = cpool.tile([F, NL2], F32)
            nc.gpsimd.dma_start(b2t[:], b2[:])
            dbt = cpool.tile([ND1, 1], F32)
            nc.gpsimd.dma_start(dbt[:], db[:])

            state = {}

            def conv_pair(a, TA, TB, H, it):
                # two row-band-disjoint positions share one [128,1024] psum
                # tile and a single strided evacuation into H.  PSUM half 0
                # always holds the lower position so the H stride stays
                # positive regardless of issue order.
                pi, pj = CONV_ORDER[2 * a], CONV_ORDER[2 * a + 1]
                lo, hi = min(pi, pj), max(pi, pj)
                ps = psC.tile([128, 2 * NB], F32, tag="C", name=f"pC{it}_{a}")
                for p in (pi, pj):
                    src, base, q0 = CONV_GEO[p]
                    K = q0 + 27 - base
                    T = TA if src == 0 else TB
                    half = 0 if p == lo else 1
                    nc.tensor.matmul(
                        ps[:, half * NB:(half + 1) * NB],
                        wcpt[base:base + K, p * F:(p + 1) * F],
                        T[base:base + K, :],
                        start=True, stop=True, tile_position=(base, 0))
                hdst = H[:, lo * NB:lo * NB + 1].copy()
                hdst.ap = hdst.ap[:1] + [[(hi - lo) * NB, 2], [1, NB]]
                if a % 2 == 0:
                    nc.scalar.activation(hdst, ps[:], AF.Relu, bias=cbt[:])
                else:
                    nc.vector.tensor_scalar(hdst, ps[:], cbt[:], 0.0,
                                            op0=ALU.add, op1=ALU.max)

            def lc1_triple(l, H, M, it):
                ps = psL.tile([128, NB], F32, tag="L", name=f"pL{it}_{l}")
                for k in range(3):
                    nc.tensor.matmul(
                        ps[:], w1t[:, (l * 3 + k) * F:(l * 3 + k + 1) * F],
                        H[:, (l + k) * NB:(l + k + 1) * NB],
                        start=(k == 0), stop=(k == 2))
                t = l // 2
                if l % 2 == 0:
                    EO = epool.tile([128, NB], F16, tag="E",
                                    name=f"E{it}_{t}")
                    nc.scalar.activation(EO[:], ps[:], AF.Relu,
                                         bias=b1t[:, l:l + 1])
                    state[("eo", t)] = EO
                else:
                    EO = state.pop(("eo", t))
                    nc.vector.scalar_tensor_tensor(
                        M[:, t * NB:(t + 1) * NB], ps[:], b1t[:, l:l + 1],
                        EO[:], op0=ALU.add, op1=ALU.max)

            def lc2_mean(M, S2, it, last=False):
                # lc2 + global-average fold.  Zero-bias path fuses the mean
                # into the evacuations: A_l = relu(ps_l) for l=0,1 (ACT),
                # then DVE scalar_tensor_tensor accumulates relu(ps_{l+2})
                # on top, and one DVE add produces Ssum.
                pss = []
                for l in range(NL2):
                    ps = psL.tile([128, NB], F32, tag="L", name=f"pT{it}_{l}")
                    for k in range(3):
                        nc.tensor.matmul(
                            ps[:], w2t[:, (l * 3 + k) * F:(l * 3 + k + 1) * F],
                            M[:, (l + k) * NB:(l + k + 1) * NB],
                            start=(k == 0), stop=(k == 2))
                    pss.append(ps)
                Ssum = sspool.tile([128, NB], F16, tag="S", name=f"Ss{it}")
                if lc2_bias_zero:
                    A = sspool.tile([128, 2 * NB], F16, tag="A", name=f"A{it}")
                    for h in range(2):
                        nc.scalar.activation(A[:, h * NB:(h + 1) * NB],
                                             pss[h][:], AF.Relu)
                        nc.vector.scalar_tensor_tensor(
                            S2[:, h * NB:(h + 1) * NB], pss[h + 2][:], 0.0,
                            A[:, h * NB:(h + 1) * NB],
                            op0=ALU.max, op1=ALU.add)
                    if last:
                        state[("s2half", it)] = S2
                        return
                    nc.vector.tensor_tensor(Ssum[:], S2[:, 0:NB], S2[:, NB:2 * NB],
                                            op=ALU.add)
                else:
                    for l in range(NL2):
                        sdst = S2[:, l * NB:(l + 1) * NB]
                        if l % 2 == 0:
                            nc.scalar.activation(sdst, pss[l][:], AF.Relu,
                                                 bias=b2t[:, l:l + 1])
                        else:
                            nc.vector.tensor_scalar(
                                sdst, pss[l][:], b2t[:, l:l + 1], 0.0,
                                op0=ALU.add, op1=ALU.max)
                    A = sspool.tile([128, 2 * NB], F16, tag="A", name=f"A{it}")
                    nc.gpsimd.tensor_tensor(A[:, 0:NB], S2[:, 0:NB],
                                            S2[:, NB:2 * NB], op=ALU.add)
                    nc.gpsimd.tensor_tensor(A[:, NB:2 * NB],
                                            S2[:, 2 * NB:3 * NB],
                                            S2[:, 3 * NB:4 * NB], op=ALU.add)
                    if last:
                        state[("s2half", it)] = A
                        return
                    nc.vector.tensor_tensor(Ssum[:], A[:, 0:NB],
                                            A[:, NB:2 * NB], op=ALU.add)
                state[("ss", it)] = Ssum

            def tail_a(it, last=False):
                # d1 matmul (mean already folded: wd1 pre-scaled by 1/4).
                # For the final tile there is no next-tile conv to hide the
                # mean chain behind, so accumulate the four S2 blocks on the
                # PE instead (S2 evacs are ready much earlier than Ssum).
                pD = psL.tile([128, NB], F32, tag="L", name=f"pD{it}")
                if last:
                    Shalf = state.pop(("s2half", it))
                    for h in range(2):
                        nc.tensor.matmul(pD[0:ND1, :], wd1t[:],
                                         Shalf[:, h * NB:(h + 1) * NB],
                                         start=(h == 0), stop=(h == 1))
                else:
                    Ssum = state.pop(("ss", it))
                    nc.tensor.matmul(pD[0:ND1, :], wd1t[:], Ssum[:],
                                     start=True, stop=True)
                state[("pd", it)] = pD

            def tail_b(it):
                pD = state.pop(("pd", it))
                S3 = s3pool.tile([ND1, NB], F16, tag="S3", name=f"S3{it}")
                nc.scalar.activation(S3[:], pD[0:ND1, :], AF.Relu,
                                     bias=dbt[:])
                # d2 reuses row 0 of the same bank; the WAR on d1's region is
                # already ordered by the S3 evac that d2 depends on.
                nc.tensor.matmul(pD[0:1, :], wd2t[:], S3[:],
                                 start=True, stop=True)
                Ys = ypool.tile([1, NB], F32, tag="Y", name=f"Y{it}")
                nc.scalar.activation(Ys[:], pD[0:1, :], AF.Sigmoid)
                nc.sync.dma_start(y[it * NB:(it + 1) * NB], Ys[0:1, :])

            for it in range(nt):
                TA, TB = T_pre.pop(it)
                if it + 1 < nt:
                    T_pre[it + 1] = load_T(it + 1)
                H = hpool.tile([128, NPOS * NB], F16, tag="H", name=f"H{it}")
                M = mpool.tile([128, NPOOL * NB], F16, tag="M", name=f"M{it}")
                S2 = s2pool.tile([128, NL2 * NB], F16, tag="S2",
                                 name=f"S2{it}")

                # conv pairs + lc1 triples interleaved; the previous tile's
                # serial tail chain (mean->d1->relu->d2) hides behind convs.
                conv_pair(0, TA, TB, H, it)    # p0, p8
                conv_pair(1, TA, TB, H, it)    # p1, p9
                conv_pair(2, TA, TB, H, it)    # p2, p10
                conv_pair(3, TA, TB, H, it)    # p5, p11
                lc1_triple(0, H, M, it)
                if it > 0:
                    tail_a(it - 1)
                lc1_triple(8, H, M, it)
                if it > 0:
                    tail_b(it - 1)
                conv_pair(4, TA, TB, H, it)    # p4, p12
                lc1_triple(9, H, M, it)
                conv_pair(5, TA, TB, H, it)    # p3, p13
                lc1_triple(10, H, M, it)
                conv_pair(6, TA, TB, H, it)    # p6, p7
                lc1_triple(11, H, M, it)
                for l in (1, 2, 3, 4, 5, 6, 7):
                    lc1_triple(l, H, M, it)
                lc2_mean(M, S2, it, last=(it == nt - 1))

            tail_a(nt - 1, last=True)
            tail_b(nt - 1)

    nc.compile()
    return nc


@functools.lru_cache(maxsize=4)
def _get_program(nt, lc2_bias_zero=True):
    return _build_program(nt, lc2_bias_zero)


def _prep_in_maps(inputs, conv_w, conv_b, lc1_w, lc1_b, lc2_w, lc2_b,
                  d1_w, d1_b, d2_w, nt=NT, n_cores=N_CORES):
    bc = nt * NB
    lc2_bias_zero = not np.any(lc2_b)
    f16, f32 = np.float16, np.float32

    wc = np.asarray(conv_w, dtype=f32).reshape(27, F)
    wcp_np = np.zeros((128, NPOS * F), dtype=f16)
    for p, (_, _, q0) in enumerate(CONV_GEO):
        wcp_np[q0:q0 + 27, p * F:(p + 1) * F] = wc
    w1_np = np.ascontiguousarray(
        np.asarray(lc1_w[:NL1], dtype=f32).reshape(NL1, 3, F, F)
        .transpose(2, 0, 1, 3).reshape(128, NL1 * 3 * F)).astype(f16)
    w2_np = np.ascontiguousarray(
        np.asarray(lc2_w, dtype=f32).reshape(NL2, 3, F, F)
        .transpose(2, 0, 1, 3).reshape(128, NL2 * 3 * F)).astype(f16)
    wd1_np = (np.asarray(d1_w, dtype=f32) * 0.25).astype(f16)
    wd2_np = np.asarray(d2_w, dtype=f32).reshape(ND1, 1).astype(f16)
    cb_np = np.ascontiguousarray(conv_b.reshape(F, 1), dtype=f32)
    b1_np = np.ascontiguousarray(np.asarray(lc1_b[:NL1], dtype=f32).T)
    b2_np = np.ascontiguousarray(np.asarray(lc2_b, dtype=f32).T)
    db_np = np.ascontiguousarray(d1_b.reshape(ND1, 1), dtype=f32)
    shared = dict(wcp=wcp_np, w1=w1_np, w2=w2_np, wd1=wd1_np, wd2=wd2_np,
                  cb=cb_np, b1=b1_np, b2=b2_np, db=db_np)

    x16 = np.asarray(inputs, dtype=f16).reshape(n_cores, bc * FEAT)
    in_maps = [dict(shared, x=x16[c]) for c in range(n_cores)]
    return in_maps, lc2_bias_zero


def kernel(inputs, conv_w, conv_b, lc1_w, lc1_b, lc2_w, lc2_b,
           d1_w, d1_b, d2_w):
    from concourse.bass_utils import run_bass_kernel_spmd

    in_maps, lc2_bias_zero = _prep_in_maps(
        inputs, conv_w, conv_b, lc1_w, lc1_b, lc2_w, lc2_b, d1_w, d1_b, d2_w)
    nc = _get_program(NT, lc2_bias_zero)
    res = run_bass_kernel_spmd(nc, in_maps, list(range(N_CORES)))
    out = np.concatenate([res.results[c]["y"] for c in range(N_CORES)])
    return out.reshape(B_FULL, 1).astype(np.float32)


# revision 28
# speedup vs baseline: 1.0757x; 1.0415x over previous
"""Trainium2 Bass kernel for nn_BCErrorCNN (dense_cnn), v3.

Network (per sample, input [17, 9]):
  Conv1D(128, k=3, relu) -> [15, 128]   (position 14 dead: never consumed)
  LocallyConnected1D(128, k=3, relu) -> [13, 128]  (position 12 dead)
  MaxPool1D(2) -> [6, 128]
  LocallyConnected1D(128, k=3, relu) -> [4, 128]
  GlobalAvgPool -> [128]; Dense(100, relu); Dense(1, sigmoid)

Sharding: pure data parallelism, batch 32768 -> 8 cores x 4096.

Fully fp16 datapath (PSUM accumulation fp32); measured rel err ~3e-4.
  - X transposed to [feature, batch] by the DMA XBAR straight out of DRAM:
    one [512,128]->[128,512] transpose per TA/TB per tile, both issued on
    the otherwise-idle SP (sync) engine so the ACT engine keeps its whole
    budget for PSUM evacuation.
  - conv reads TA/TB directly with zero-padded weights at legal 32-aligned
    base partitions (no strip DMAs); issue order alternates PE row bands
    so row-disjoint conv matmuls overlap in the array.
  - conv matmuls + lc1 triples interleaved in PE issue order so PSUM evac
    latency hides behind matmul work; conv PSUM singles with bufs=3.
  - lc1 evac fused with maxpool: ACT relu-evacs the even position, DVE
    scalar_tensor_tensor computes max(odd+bias, relu(even)) which equals
    relu(max(even+b, odd+b)) since relu(x) >= 0.
  - global-average-pool folded in front of Dense(100): S2 position sums
    on GPSIMD+DVE (wd1 pre-scaled by 1/4), so d1 is ONE matmul per tile.
  - d2 writes PSUM partition 96 (32-aligned PE column tile) so the tail
    needs only one [128,512] PSUM bank; per-tile sigmoid + output DMA.
"""

import functools

import numpy as np

# ---- constants (hardcoded per problem spec) --------------------------------
N_CORES = 8
B_FULL = 32768
BC = B_FULL // N_CORES  # per-core batch
NB = 512                # batch tile (columns per matmul)
NT = BC // NB           # batch tiles per core
LIN, CIN, F = 17, 9, 128
FEAT = LIN * CIN        # 153
NPOS = 14               # conv positions actually needed (0..13)
NL1 = 12                # lc1 positions needed (0..11)
NPOOL = 6
NL2 = 4
ND1 = 100

# Conv position p contracts feature rows 9p..9p+26.  TA holds features
# 0..127 on partitions 0..127, TB holds features 25..152.  The matmul
# base-partition rule constrains tile_position[0] by contraction size K:
# K<=32 -> {0,32,64,96}; K<=64 -> {0,64}; else 0.  q0 below is the
# partition where wc row 0 sits (TA: 9p; TB: 9p-25), base is the chosen
# 32-aligned start, K = q0 + 27 - base.
CONV_GEO = [
    (0, 0, 0), (0, 0, 9), (0, 0, 18), (0, 0, 27),      # p0..p3
    (0, 32, 36), (0, 0, 45), (0, 0, 54), (0, 0, 63),   # p4..p7
    (0, 64, 72), (0, 64, 81), (0, 64, 90), (0, 96, 99),  # p8..p11
    (1, 64, 83), (1, 64, 92),                          # p12, p13 (TB)
]
# Issue order: consecutive matmuls sit in disjoint PE row ranges where
# possible so the systolic array overlaps them.
CONV_ORDER = [0, 8, 1, 9, 2, 10, 5, 11, 4, 12, 3, 13, 6, 7]


def _build_program(nt=NT, lc2_bias_zero=True):
    import concourse.tile as tile
    from concourse import bacc, mybir

    F32 = mybir.dt.float32
    F16 = mybir.dt.float16
    AF = mybir.ActivationFunctionType
    ALU = mybir.AluOpType

    bc = nt * NB
    nc = bacc.Bacc("TRN2", target_bir_lowering=False, debug=False,
                   num_devices=N_CORES)

    x = nc.dram_tensor("x", [bc * FEAT], F16, kind="ExternalInput").ap()
    wcp = nc.dram_tensor("wcp", [128, NPOS * F], F16, kind="ExternalInput").ap()
    w1 = nc.dram_tensor("w1", [128, NL1 * 3 * F], F16, kind="ExternalInput").ap()
    w2 = nc.dram_tensor("w2", [128, NL2 * 3 * F], F16, kind="ExternalInput").ap()
    wd1 = nc.dram_tensor("wd1", [F, ND1], F16, kind="ExternalInput").ap()
    wd2 = nc.dram_tensor("wd2", [ND1, 1], F16, kind="ExternalInput").ap()
    cb = nc.dram_tensor("cb", [F, 1], F32, kind="ExternalInput").ap()
    b1 = nc.dram_tensor("b1", [F, NL1], F32, kind="ExternalInput").ap()
    b2 = nc.dram_tensor("b2", [F, NL2], F32, kind="ExternalInput").ap()
    db = nc.dram_tensor("db", [ND1, 1], F32, kind="ExternalInput").ap()
    y = nc.dram_tensor("y", [bc], F32, kind="ExternalOutput").ap()

    with tile.TileContext(nc) as tc:
        with (
            tc.tile_pool(name="const", bufs=1) as cpool,
            tc.tile_pool(name="t", bufs=2) as tpool,
            tc.tile_pool(name="h", bufs=2) as hpool,
            tc.tile_pool(name="eo", bufs=3) as epool,
            tc.tile_pool(name="m", bufs=2) as mpool,
            tc.tile_pool(name="s2", bufs=2) as s2pool,
            tc.tile_pool(name="ss", bufs=2) as sspool,
            tc.tile_pool(name="s3", bufs=2) as s3pool,
            tc.tile_pool(name="ys", bufs=2) as ypool,
            tc.tile_pool(name="psC", bufs=2, space="PSUM") as psC,
            tc.tile_pool(name="psL", bufs=4, space="PSUM") as psL,
        ):
            def load_T(jt):
                # DMA-XBAR transpose: DRAM [batch, feat] -> SBUF [feat, batch]
                # One [512,128]->[128,512] instruction per target, on sync
                # (tile 0's TB goes on scalar so TA/TB land in parallel).
                TA = tpool.tile([128, NB], F16, tag="TA", name=f"TA{jt}")
                TB = tpool.tile([128, NB], F16, tag="TB", name=f"TB{jt}")
                b0 = jt * NB * FEAT
                srcA = x[b0:b0 + 1].copy()
                srcA.ap = srcA.ap[:0] + [[FEAT, NB], [1, 128]]
                nc.sync.dma_start(TA[:], srcA, transpose=True)
                srcB = x[b0 + 25:b0 + 26].copy()
                srcB.ap = srcB.ap[:0] + [[FEAT, NB], [1, 128]]
                eng = nc.scalar if jt == 0 else nc.sync
                eng.dma_start(TB[:], srcB, transpose=True)
                return TA, TB

            T_pre = {0: load_T(0)}

            # ---- weights + biases: all on the gpsimd software DGE so the
            # hwdge (sync/scalar) DMA semaphores that conv/lc matmuls wait
            # on count only transposes, never multi-us weight transfers.
            # Order = first-use order: early conv weight blocks and the
            # biases the first evacuations read come before the bulk.
            wcpt = cpool.tile([128, NPOS * F], F16)
            nc.gpsimd.dma_start(wcpt[:, :11 * F], wcp[:, :11 * F])
            cbt = cpool.tile([F, 1], F32)
            nc.gpsimd.dma_start(cbt[:], cb[:])
            b1t = cpool.tile([F, NL1], F32)
            nc.gpsimd.dma_start(b1t[:], b1[:])
            nc.gpsimd.dma_start(wcpt[:, 11 * F:], wcp[:, 11 * F:])
            w1t = cpool.tile([128, NL1 * 3 * F], F16)
            nc.gpsimd.dma_start(w1t[:, :NL1 * 3 * F // 2],
                                w1[:, :NL1 * 3 * F // 2])
            nc.gpsimd.dma_start(w1t[:, NL1 * 3 * F // 2:],
                                w1[:, NL1 * 3 * F // 2:])
            w2t = cpool.tile([128, NL2 * 3 * F], F16)
            nc.gpsimd.dma_start(w2t[:], w2[:])
            wd1t = cpool.tile([F, ND1], F16)
            nc.gpsimd.dma_start(wd1t[:], wd1[:])
            wd2t = cpool.tile([ND1, 1], F16)
            nc.gpsimd.dma_start(wd2t[:], wd2[:])
            b2t = cpool.tile([F, NL2], F32)
            nc.gpsimd.dma_start(b2t[:], b2[:])
            dbt = cpool.tile([ND1, 1], F32)
            nc.gpsimd.dma_start(dbt[:], db[:])

            state = {}

            def conv_pair(a, TA, TB, H, it):
                # two row-band-disjoint positions share one [128,1024] psum
                # tile and a single strided evacuation into H.  PSUM half 0
                # always holds the lower position so the H stride stays
                # positive regardless of issue order.
                pi, pj = CONV_ORDER[2 * a], CONV_ORDER[2 * a + 1]
                lo, hi = min(pi, pj), max(pi, pj)
                ps = psC.tile([128, 2 * NB], F32, tag="C", name=f"pC{it}_{a}")
                for p in (pi, pj):
                    src, base, q0 = CONV_GEO[p]
                    K = q0 + 27 - base
                    T = TA if src == 0 else TB
                    half = 0 if p == lo else 1
                    nc.tensor.matmul(
                        ps[:, half * NB:(half + 1) * NB],
                        wcpt[base:base + K, p * F:(p + 1) * F],
                        T[base:base + K, :],
                        start=True, stop=True, tile_position=(base, 0))
                hdst = H[:, lo * NB:lo * NB + 1].copy()
                hdst.ap = hdst.ap[:1] + [[(hi - lo) * NB, 2], [1, NB]]
                if a % 2 == 0:
                    nc.scalar.activation(hdst, ps[:], AF.Relu, bias=cbt[:])
                else:
                    nc.vector.tensor_scalar(hdst, ps[:], cbt[:], 0.0,
                                            op0=ALU.add, op1=ALU.max)

            def lc1_triple(l, H, M, it):
                ps = psL.tile([128, NB], F32, tag="L", name=f"pL{it}_{l}")
                for k in range(3):
                    nc.tensor.matmul(
                        ps[:], w1t[:, (l * 3 + k) * F:(l * 3 + k + 1) * F],
                        H[:, (l + k) * NB:(l + k + 1) * NB],
                        start=(k == 0), stop=(k == 2))
                t = l // 2
                if l % 2 == 0:
                    EO = epool.tile([128, NB], F16, tag="E",
                                    name=f"E{it}_{t}")
                    nc.scalar.activation(EO[:], ps[:], AF.Relu,
                                         bias=b1t[:, l:l + 1])
                    state[("eo", t)] = EO
                else:
                    EO = state.pop(("eo", t))
                    nc.vector.scalar_tensor_tensor(
                        M[:, t * NB:(t + 1) * NB], ps[:], b1t[:, l:l + 1],
                        EO[:], op0=ALU.add, op1=ALU.max)

            def lc2_mean(M, S2, it, last=False):
                # lc2 + global-average fold.  Zero-bias path fuses the mean
                # into the evacuations: A_l = relu(ps_l) for l=0,1 (ACT),
                # then DVE scalar_tensor_tensor accumulates relu(ps_{l+2})
                # on top, and one DVE add produces Ssum.
                pss = []
                for l in range(NL2):
                    ps = psL.tile([128, NB], F32, tag="L", name=f"pT{it}_{l}")
                    for k in range(3):
                        nc.tensor.matmul(
                            ps[:], w2t[:, (l * 3 + k) * F:(l * 3 + k + 1) * F],
                            M[:, (l + k) * NB:(l + k + 1) * NB],
                            start=(k == 0), stop=(k == 2))
                    pss.append(ps)
                if lc2_bias_zero:
                    A = sspool.tile([128, 2 * NB], F16, tag="A", name=f"A{it}")
                    for h in range(2):
                        nc.scalar.activation(A[:, h * NB:(h + 1) * NB],
                                             pss[h][:], AF.Relu)
                        nc.vector.scalar_tensor_tensor(
                            S2[:, h * NB:(h + 1) * NB], pss[h + 2][:], 0.0,
                            A[:, h * NB:(h + 1) * NB],
                            op0=ALU.max, op1=ALU.add)
                    if last:
                        state[("s2half", it)] = S2
                        return
                    Ssum = sspool.tile([128, NB], F16, tag="S", name=f"Ss{it}")
                    nc.vector.tensor_tensor(Ssum[:], S2[:, 0:NB], S2[:, NB:2 * NB],
                                            op=ALU.add)
                else:
                    for l in range(NL2):
                        sdst = S2[:, l * NB:(l + 1) * NB]
                        if l % 2 == 0:
                            nc.scalar.activation(sdst, pss[l][:], AF.Relu,
                                                 bias=b2t[:, l:l + 1])
                        else:
                            nc.vector.tensor_scalar(
                                sdst, pss[l][:], b2t[:, l:l + 1], 0.0,
                                op0=ALU.add, op1=ALU.max)
                    A = sspool.tile([128, 2 * NB], F16, tag="A", name=f"A{it}")
                    nc.gpsimd.tensor_tensor(A[:, 0:NB], S2[:, 0:NB],
                                            S2[:, NB:2 * NB], op=ALU.add)
                    nc.gpsimd.tensor_tensor(A[:, NB:2 * NB],
                                            S2[:, 2 * NB:3 * NB],
                                            S2[:, 3 * NB:4 * NB], op=ALU.add)
                    if last:
                        state[("s2half", it)] = A
                        return
                    Ssum = sspool.tile([128, NB], F16, tag="S", name=f"Ss{it}")
                    nc.vector.tensor_tensor(Ssum[:], A[:, 0:NB],
                                            A[:, NB:2 * NB], op=ALU.add)
                state[("ss", it)] = Ssum

            def tail_a(it, last=False):
                # d1 matmul (mean already folded: wd1 pre-scaled by 1/4).
                # For the final tile there is no next-tile conv to hide the
                # mean chain behind, so accumulate the four S2 blocks on the
                # PE instead (S2 evacs are ready much earlier than Ssum).
                pD = psL.tile([128, NB], F32, tag="L", name=f"pD{it}")
                if last:
                    Shalf = state.pop(("s2half", it))
                    for h in range(2):
                        nc.tensor.matmul(pD[0:ND1, :], wd1t[:],
                                         Shalf[:, h * NB:(h + 1) * NB],
                                         start=(h == 0), stop=(h == 1))
                else:
                    Ssum = state.pop(("ss", it))
                    nc.tensor.matmul(pD[0:ND1, :], wd1t[:], Ssum[:],
                                     start=True, stop=True)
                state[("pd", it)] = pD

            def tail_b(it):
                pD = state.pop(("pd", it))
                S3 = s3pool.tile([ND1, NB], F16, tag="S3", name=f"S3{it}")
                nc.scalar.activation(S3[:], pD[0:ND1, :], AF.Relu,
                                     bias=dbt[:])
                # d2 reuses row 0 of the same bank; the WAR on d1's region is
                # already ordered by the S3 evac that d2 depends on.
                nc.tensor.matmul(pD[0:1, :], wd2t[:], S3[:],
                                 start=True, stop=True)
                Ys = ypool.tile([1, NB], F32, tag="Y", name=f"Y{it}")
                nc.scalar.activation(Ys[:], pD[0:1, :], AF.Sigmoid)
                nc.sync.dma_start(y[it * NB:(it + 1) * NB], Ys[0:1, :])

            for it in range(nt):
                TA, TB = T_pre.pop(it)
                if it + 1 < nt:
                    T_pre[it + 1] = load_T(it + 1)
                H = hpool.tile([128, NPOS * NB], F16, tag="H", name=f"H{it}")
                M = mpool.tile([128, NPOOL * NB], F16, tag="M", name=f"M{it}")
                S2 = s2pool.tile([128, NL2 * NB], F16, tag="S2",
                                 name=f"S2{it}")

                # conv pairs + lc1 triples interleaved; the previous tile's
                # serial tail chain (mean->d1->relu->d2) hides behind convs.
                conv_pair(0, TA, TB, H, it)    # p0, p8
                conv_pair(1, TA, TB, H, it)    # p1, p9
                conv_pair(2, TA, TB, H, it)    # p2, p10
                conv_pair(3, TA, TB, H, it)    # p5, p11
                lc1_triple(0, H, M, it)
                if it > 0:
                    tail_a(it - 1)
                lc1_triple(8, H, M, it)
                if it > 0:
                    tail_b(it - 1)
                conv_pair(4, TA, TB, H, it)    # p4, p12
                lc1_triple(9, H, M, it)
                conv_pair(5, TA, TB, H, it)    # p3, p13
                lc1_triple(10, H, M, it)
                conv_pair(6, TA, TB, H, it)    # p6, p7
                lc1_triple(11, H, M, it)
                for l in (1, 2, 3, 4, 5, 6, 7):
                    lc1_triple(l, H, M, it)
                lc2_mean(M, S2, it, last=(it == nt - 1))

            tail_a(nt - 1, last=True)
            tail_b(nt - 1)

    nc.compile()
    return nc


@functools.lru_cache(maxsize=4)
def _get_program(nt, lc2_bias_zero=True):
    return _build_program(nt, lc2_bias_zero)


def _prep_in_maps(inputs, conv_w, conv_b, lc1_w, lc1_b, lc2_w, lc2_b,
                  d1_w, d1_b, d2_w, nt=NT, n_cores=N_CORES):
    bc = nt * NB
    lc2_bias_zero = not np.any(lc2_b)
    f16, f32 = np.float16, np.float32

    wc = np.asarray(conv_w, dtype=f32).reshape(27, F)
    wcp_np = np.zeros((128, NPOS * F), dtype=f16)
    for p, (_, _, q0) in enumerate(CONV_GEO):
        wcp_np[q0:q0 + 27, p * F:(p + 1) * F] = wc
    w1_np = np.ascontiguousarray(
        np.asarray(lc1_w[:NL1], dtype=f32).reshape(NL1, 3, F, F)
        .transpose(2, 0, 1, 3).reshape(128, NL1 * 3 * F)).astype(f16)
    w2_np = np.ascontiguousarray(
        np.asarray(lc2_w, dtype=f32).reshape(NL2, 3, F, F)
        .transpose(2, 0, 1, 3).reshape(128, NL2 * 3 * F)).astype(f16)
    wd1_np = (np.asarray(d1_w, dtype=f32) * 0.25).astype(f16)
    wd2_np = np.asarray(d2_w, dtype=f32).reshape(ND1, 1).astype(f16)
    cb_np = np.ascontiguousarray(conv_b.reshape(F, 1), dtype=f32)
    b1_np = np.ascontiguousarray(np.asarray(lc1_b[:NL1], dtype=f32).T)
    b2_np = np.ascontiguousarray(np.asarray(lc2_b, dtype=f32).T)
    db_np = np.ascontiguousarray(d1_b.reshape(ND1, 1), dtype=f32)
    shared = dict(wcp=wcp_np, w1=w1_np, w2=w2_np, wd1=wd1_np, wd2=wd2_np,
                  cb=cb_np, b1=b1_np, b2=b2_np, db=db_np)

    x16 = np.asarray(inputs, dtype=f16).reshape(n_cores, bc * FEAT)
    in_maps = [dict(shared, x=x16[c]) for c in range(n_cores)]
    return in_maps, lc2_bias_zero


def kernel(inputs, conv_w, conv_b, lc1_w, lc1_b, lc2_w, lc2_b,
           d1_w, d1_b, d2_w):
    from concourse.bass_utils import run_bass_kernel_spmd

    in_maps, lc2_bias_zero = _prep_in_maps(
        inputs, conv_w, conv_b, lc1_w, lc1_b, lc2_w, lc2_b, d1_w, d1_b, d2_w)
    nc = _get_program(NT, lc2_bias_zero)
    res = run_bass_kernel_spmd(nc, in_maps, list(range(N_CORES)))
    out = np.concatenate([res.results[c]["y"] for c in range(N_CORES)])
    return out.reshape(B_FULL, 1).astype(np.float32)


# revision 29
# speedup vs baseline: 1.1079x; 1.0300x over previous
"""Trainium2 Bass kernel for nn_BCErrorCNN (dense_cnn), v3.

Network (per sample, input [17, 9]):
  Conv1D(128, k=3, relu) -> [15, 128]   (position 14 dead: never consumed)
  LocallyConnected1D(128, k=3, relu) -> [13, 128]  (position 12 dead)
  MaxPool1D(2) -> [6, 128]
  LocallyConnected1D(128, k=3, relu) -> [4, 128]
  GlobalAvgPool -> [128]; Dense(100, relu); Dense(1, sigmoid)

Sharding: pure data parallelism, batch 32768 -> 8 cores x 4096.

Fully fp16 datapath (PSUM accumulation fp32); measured rel err ~3e-4.
  - X transposed to [feature, batch] by the DMA XBAR straight out of DRAM:
    one [512,128]->[128,512] transpose per TA/TB per tile, both issued on
    the otherwise-idle SP (sync) engine so the ACT engine keeps its whole
    budget for PSUM evacuation.
  - conv reads TA/TB directly with zero-padded weights at legal 32-aligned
    base partitions (no strip DMAs); issue order alternates PE row bands
    so row-disjoint conv matmuls overlap in the array.
  - conv matmuls + lc1 triples interleaved in PE issue order so PSUM evac
    latency hides behind matmul work; conv PSUM singles with bufs=3.
  - lc1 evac fused with maxpool: ACT relu-evacs the even position, DVE
    scalar_tensor_tensor computes max(odd+bias, relu(even)) which equals
    relu(max(even+b, odd+b)) since relu(x) >= 0.
  - global-average-pool folded in front of Dense(100): S2 position sums
    on GPSIMD+DVE (wd1 pre-scaled by 1/4), so d1 is ONE matmul per tile.
  - d2 writes PSUM partition 96 (32-aligned PE column tile) so the tail
    needs only one [128,512] PSUM bank; per-tile sigmoid + output DMA.
"""

import functools

import numpy as np

# ---- constants (hardcoded per problem spec) --------------------------------
N_CORES = 8
B_FULL = 32768
BC = B_FULL // N_CORES  # per-core batch
NB = 512                # batch tile (columns per matmul)
NT = BC // NB           # batch tiles per core
LIN, CIN, F = 17, 9, 128
FEAT = LIN * CIN        # 153
NPOS = 14               # conv positions actually needed (0..13)
NL1 = 12                # lc1 positions needed (0..11)
NPOOL = 6
NL2 = 4
ND1 = 100

# Conv position p contracts feature rows 9p..9p+26.  TA holds features
# 0..127 on partitions 0..127, TB holds features 25..152.  The matmul
# base-partition rule constrains tile_position[0] by contraction size K:
# K<=32 -> {0,32,64,96}; K<=64 -> {0,64}; else 0.  q0 below is the
# partition where wc row 0 sits (TA: 9p; TB: 9p-25), base is the chosen
# 32-aligned start, K = q0 + 27 - base.
CONV_GEO = [
    (0, 0, 0), (0, 0, 9), (0, 0, 18), (0, 0, 27),      # p0..p3
    (0, 32, 36), (0, 0, 45), (0, 0, 54), (0, 0, 63),   # p4..p7
    (0, 64, 72), (0, 64, 81), (0, 64, 90), (0, 96, 99),  # p8..p11
    (1, 64, 83), (1, 64, 92),                          # p12, p13 (TB)
]
# Issue order: consecutive matmuls sit in disjoint PE row ranges where
# possible so the systolic array overlaps them.
CONV_ORDER = [0, 8, 1, 9, 2, 10, 5, 11, 4, 12, 3, 13, 6, 7]


def _build_program(nt=NT, lc2_bias_zero=True):
    import concourse.tile as tile
    from concourse import bacc, mybir

    F32 = mybir.dt.float32
    F16 = mybir.dt.float16
    AF = mybir.ActivationFunctionType
    ALU = mybir.AluOpType

    bc = nt * NB
    nc = bacc.Bacc("TRN2", target_bir_lowering=False, debug=False,
                   num_devices=N_CORES)

    x = nc.dram_tensor("x", [bc * FEAT], F16, kind="ExternalInput").ap()
    wcp = nc.dram_tensor("wcp", [128, NPOS * F], F16, kind="ExternalInput").ap()
    w1 = nc.dram_tensor("w1", [128, NL1 * 3 * F], F16, kind="ExternalInput").ap()
    w2 = nc.dram_tensor("w2", [128, NL2 * 3 * F], F16, kind="ExternalInput").ap()
    wd1 = nc.dram_tensor("wd1", [F, ND1], F16, kind="ExternalInput").ap()
    wd2 = nc.dram_tensor("wd2", [ND1, 1], F16, kind="ExternalInput").ap()
    cb = nc.dram_tensor("cb", [F, 1], F32, kind="ExternalInput").ap()
    b1 = nc.dram_tensor("b1", [F, NL1], F32, kind="ExternalInput").ap()
    b2 = nc.dram_tensor("b2", [F, NL2], F32, kind="ExternalInput").ap()
    db = nc.dram_tensor("db", [ND1, 1], F32, kind="ExternalInput").ap()
    y = nc.dram_tensor("y", [bc], F32, kind="ExternalOutput").ap()

    with tile.TileContext(nc) as tc:
        with (
            tc.tile_pool(name="const", bufs=1) as cpool,
            tc.tile_pool(name="t", bufs=2) as tpool,
            tc.tile_pool(name="h", bufs=2) as hpool,
            tc.tile_pool(name="eo", bufs=3) as epool,
            tc.tile_pool(name="m", bufs=2) as mpool,
            tc.tile_pool(name="s2", bufs=2) as s2pool,
            tc.tile_pool(name="ss", bufs=2) as sspool,
            tc.tile_pool(name="s3", bufs=2) as s3pool,
            tc.tile_pool(name="ys", bufs=2) as ypool,
            tc.tile_pool(name="psC", bufs=2, space="PSUM") as psC,
            tc.tile_pool(name="psL", bufs=4, space="PSUM") as psL,
        ):
            def load_T(jt):
                # DMA-XBAR transpose: DRAM [batch, feat] -> SBUF [feat, batch]
                # One [512,128]->[128,512] instruction per target, on sync
                # (tile 0's TB goes on scalar so TA/TB land in parallel).
                TA = tpool.tile([128, NB], F16, tag="TA", name=f"TA{jt}")
                TB = tpool.tile([128, NB], F16, tag="TB", name=f"TB{jt}")
                b0 = jt * NB * FEAT
                srcA = x[b0:b0 + 1].copy()
                srcA.ap = srcA.ap[:0] + [[FEAT, NB], [1, 128]]
                nc.sync.dma_start(TA[:], srcA, transpose=True)
                srcB = x[b0 + 25:b0 + 26].copy()
                srcB.ap = srcB.ap[:0] + [[FEAT, NB], [1, 128]]
                eng = nc.scalar if jt == 0 else nc.sync
                eng.dma_start(TB[:], srcB, transpose=True)
                return TA, TB

            T_pre = {0: load_T(0)}

            # ---- weights + biases: all on the gpsimd software DGE so the
            # hwdge (sync/scalar) DMA semaphores that conv/lc matmuls wait
            # on count only transposes, never multi-us weight transfers.
            # Order = first-use order: early conv weight blocks and the
            # biases the first evacuations read come before the bulk.
            wcpt = cpool.tile([128, NPOS * F], F16)
            nc.gpsimd.dma_start(wcpt[:, :11 * F], wcp[:, :11 * F])
            cbt = cpool.tile([F, 1], F32)
            nc.gpsimd.dma_start(cbt[:], cb[:])
            b1t = cpool.tile([F, NL1], F32)
            nc.gpsimd.dma_start(b1t[:], b1[:])
            nc.gpsimd.dma_start(wcpt[:, 11 * F:], wcp[:, 11 * F:])
            w1t = cpool.tile([128, NL1 * 3 * F], F16)
            nc.gpsimd.dma_start(w1t[:, :NL1 * 3 * F // 2],
                                w1[:, :NL1 * 3 * F // 2])
            nc.gpsimd.dma_start(w1t[:, NL1 * 3 * F // 2:],
                                w1[:, NL1 * 3 * F // 2:])
            w2t = cpool.tile([128, NL2 * 3 * F], F16)
            nc.gpsimd.dma_start(w2t[:], w2[:])
            wd1t = cpool.tile([F, ND1], F16)
            nc.gpsimd.dma_start(wd1t[:], wd1[:])
            wd2t = cpool.tile([ND1, 1], F16)
            nc.gpsimd.dma_start(wd2t[:], wd2[:])
            b2t = cpool.tile([F, NL2], F32)
            nc.gpsimd.dma_start(b2t[:], b2[:])
            dbt = cpool.tile([ND1, 1], F32)
            nc.gpsimd.dma_start(dbt[:], db[:])

            state = {}

            def conv_pair(a, TA, TB, H, it):
                # two row-band-disjoint positions share one [128,1024] psum
                # tile and a single strided evacuation into H.  PSUM half 0
                # always holds the lower position so the H stride stays
                # positive regardless of issue order.
                pi, pj = CONV_ORDER[2 * a], CONV_ORDER[2 * a + 1]
                lo, hi = min(pi, pj), max(pi, pj)
                ps = psC.tile([128, 2 * NB], F32, tag="C", name=f"pC{it}_{a}")
                for p in (pi, pj):
                    src, base, q0 = CONV_GEO[p]
                    K = q0 + 27 - base
                    T = TA if src == 0 else TB
                    half = 0 if p == lo else 1
                    nc.tensor.matmul(
                        ps[:, half * NB:(half + 1) * NB],
                        wcpt[base:base + K, p * F:(p + 1) * F],
                        T[base:base + K, :],
                        start=True, stop=True, tile_position=(base, 0))
                hdst = H[:, lo * NB:lo * NB + 1].copy()
                hdst.ap = hdst.ap[:1] + [[(hi - lo) * NB, 2], [1, NB]]
                if a % 2 == 0:
                    nc.scalar.activation(hdst, ps[:], AF.Relu, bias=cbt[:])
                else:
                    nc.vector.tensor_scalar(hdst, ps[:], cbt[:], 0.0,
                                            op0=ALU.add, op1=ALU.max)

            def lc1_triple(l, H, M, it):
                ps = psL.tile([128, NB], F32, tag="L", name=f"pL{it}_{l}")
                for k in range(3):
                    nc.tensor.matmul(
                        ps[:], w1t[:, (l * 3 + k) * F:(l * 3 + k + 1) * F],
                        H[:, (l + k) * NB:(l + k + 1) * NB],
                        start=(k == 0), stop=(k == 2))
                t = l // 2
                if l % 2 == 0:
                    EO = epool.tile([128, NB], F16, tag="E",
                                    name=f"E{it}_{t}")
                    nc.scalar.activation(EO[:], ps[:], AF.Relu,
                                         bias=b1t[:, l:l + 1])
                    state[("eo", t)] = EO
                else:
                    EO = state.pop(("eo", t))
                    nc.vector.scalar_tensor_tensor(
                        M[:, t * NB:(t + 1) * NB], ps[:], b1t[:, l:l + 1],
                        EO[:], op0=ALU.add, op1=ALU.max)

            def lc2_mean(M, S2, it):
                # lc2 + global-average fold.  Zero-bias path fuses the mean
                # into the evacuations: A_l = relu(ps_l) for l=0,1 (ACT),
                # then DVE scalar_tensor_tensor accumulates relu(ps_{l+2})
                # on top, and one DVE add produces Ssum.
                pss = []
                for l in range(NL2):
                    ps = psL.tile([128, NB], F32, tag="L", name=f"pT{it}_{l}")
                    for k in range(3):
                        nc.tensor.matmul(
                            ps[:], w2t[:, (l * 3 + k) * F:(l * 3 + k + 1) * F],
                            M[:, (l + k) * NB:(l + k + 1) * NB],
                            start=(k == 0), stop=(k == 2))
                    pss.append(ps)
                if lc2_bias_zero:
                    A = sspool.tile([128, 2 * NB], F16, tag="A", name=f"A{it}")
                    for h in range(2):
                        nc.scalar.activation(A[:, h * NB:(h + 1) * NB],
                                             pss[h][:], AF.Relu)
                        nc.vector.scalar_tensor_tensor(
                            S2[:, h * NB:(h + 1) * NB], pss[h + 2][:], 0.0,
                            A[:, h * NB:(h + 1) * NB],
                            op0=ALU.max, op1=ALU.add)
                    state[("s2half", it)] = S2
                else:
                    for l in range(NL2):
                        sdst = S2[:, l * NB:(l + 1) * NB]
                        if l % 2 == 0:
                            nc.scalar.activation(sdst, pss[l][:], AF.Relu,
                                                 bias=b2t[:, l:l + 1])
                        else:
                            nc.vector.tensor_scalar(
                                sdst, pss[l][:], b2t[:, l:l + 1], 0.0,
                                op0=ALU.add, op1=ALU.max)
                    A = sspool.tile([128, 2 * NB], F16, tag="A", name=f"A{it}")
                    nc.gpsimd.tensor_tensor(A[:, 0:NB], S2[:, 0:NB],
                                            S2[:, NB:2 * NB], op=ALU.add)
                    nc.gpsimd.tensor_tensor(A[:, NB:2 * NB],
                                            S2[:, 2 * NB:3 * NB],
                                            S2[:, 3 * NB:4 * NB], op=ALU.add)
                    state[("s2half", it)] = A

            def tail_a(it, h):
                # d1 as two accumulating matmuls over the S2 half-sums
                # (mean folded via wd1 pre-scale): no serial mean chain,
                # each half is consumed as soon as its evac lands.
                if h == 0:
                    pD = psL.tile([128, NB], F32, tag="L", name=f"pD{it}")
                    state[("pd", it)] = pD
                else:
                    pD = state[("pd", it)]
                Shalf = state[("s2half", it)] if h == 0 else \
                    state.pop(("s2half", it))
                nc.tensor.matmul(pD[0:ND1, :], wd1t[:],
                                 Shalf[:, h * NB:(h + 1) * NB],
                                 start=(h == 0), stop=(h == 1))

            def tail_b(it):
                pD = state.pop(("pd", it))
                S3 = s3pool.tile([ND1, NB], F16, tag="S3", name=f"S3{it}")
                nc.scalar.activation(S3[:], pD[0:ND1, :], AF.Relu,
                                     bias=dbt[:])
                # d2 reuses row 0 of the same bank; the WAR on d1's region is
                # already ordered by the S3 evac that d2 depends on.
                nc.tensor.matmul(pD[0:1, :], wd2t[:], S3[:],
                                 start=True, stop=True)
                Ys = ypool.tile([1, NB], F32, tag="Y", name=f"Y{it}")
                nc.scalar.activation(Ys[:], pD[0:1, :], AF.Sigmoid)
                nc.sync.dma_start(y[it * NB:(it + 1) * NB], Ys[0:1, :])

            for it in range(nt):
                TA, TB = T_pre.pop(it)
                if it + 1 < nt:
                    T_pre[it + 1] = load_T(it + 1)
                H = hpool.tile([128, NPOS * NB], F16, tag="H", name=f"H{it}")
                M = mpool.tile([128, NPOOL * NB], F16, tag="M", name=f"M{it}")
                S2 = s2pool.tile([128, NL2 * NB], F16, tag="S2",
                                 name=f"S2{it}")

                # conv pairs + lc1 triples interleaved; the previous tile's
                # serial tail chain (mean->d1->relu->d2) hides behind convs.
                conv_pair(0, TA, TB, H, it)    # p0, p8
                conv_pair(1, TA, TB, H, it)    # p1, p9
                if it > 0:
                    tail_a(it - 1, 0)
                conv_pair(2, TA, TB, H, it)    # p2, p10
                if it > 0:
                    tail_a(it - 1, 1)
                conv_pair(3, TA, TB, H, it)    # p5, p11
                lc1_triple(0, H, M, it)
                lc1_triple(8, H, M, it)
                if it > 0:
                    tail_b(it - 1)
                conv_pair(4, TA, TB, H, it)    # p4, p12
                lc1_triple(9, H, M, it)
                conv_pair(5, TA, TB, H, it)    # p3, p13
                lc1_triple(10, H, M, it)
                conv_pair(6, TA, TB, H, it)    # p6, p7
                lc1_triple(11, H, M, it)
                for l in (1, 2, 3, 4, 5, 6, 7):
                    lc1_triple(l, H, M, it)
                lc2_mean(M, S2, it)

            tail_a(nt - 1, 0)
            tail_a(nt - 1, 1)
            tail_b(nt - 1)

    nc.compile()
    return nc


@functools.lru_cache(maxsize=4)
def _get_program(nt, lc2_bias_zero=True):
    return _build_program(nt, lc2_bias_zero)


def _prep_in_maps(inputs, conv_w, conv_b, lc1_w, lc1_b, lc2_w, lc2_b,
                  d1_w, d1_b, d2_w, nt=NT, n_cores=N_CORES):
    bc = nt * NB
    lc2_bias_zero = not np.any(lc2_b)
    f16, f32 = np.float16, np.float32

    wc = np.asarray(conv_w, dtype=f32).reshape(27, F)
    wcp_np = np.zeros((128, NPOS * F), dtype=f16)
    for p, (_, _, q0) in enumerate(CONV_GEO):
        wcp_np[q0:q0 + 27, p * F:(p + 1) * F] = wc
    w1_np = np.ascontiguousarray(
        np.asarray(lc1_w[:NL1], dtype=f32).reshape(NL1, 3, F, F)
        .transpose(2, 0, 1, 3).reshape(128, NL1 * 3 * F)).astype(f16)
    w2_np = np.ascontiguousarray(
        np.asarray(lc2_w, dtype=f32).reshape(NL2, 3, F, F)
        .transpose(2, 0, 1, 3).reshape(128, NL2 * 3 * F)).astype(f16)
    wd1_np = (np.asarray(d1_w, dtype=f32) * 0.25).astype(f16)
    wd2_np = np.asarray(d2_w, dtype=f32).reshape(ND1, 1).astype(f16)
    cb_np = np.ascontiguousarray(conv_b.reshape(F, 1), dtype=f32)
    b1_np = np.ascontiguousarray(np.asarray(lc1_b[:NL1], dtype=f32).T)
    b2_np = np.ascontiguousarray(np.asarray(lc2_b, dtype=f32).T)
    db_np = np.ascontiguousarray(d1_b.reshape(ND1, 1), dtype=f32)
    shared = dict(wcp=wcp_np, w1=w1_np, w2=w2_np, wd1=wd1_np, wd2=wd2_np,
                  cb=cb_np, b1=b1_np, b2=b2_np, db=db_np)

    x16 = np.asarray(inputs, dtype=f16).reshape(n_cores, bc * FEAT)
    in_maps = [dict(shared, x=x16[c]) for c in range(n_cores)]
    return in_maps, lc2_bias_zero


def kernel(inputs, conv_w, conv_b, lc1_w, lc1_b, lc2_w, lc2_b,
           d1_w, d1_b, d2_w):
    from concourse.bass_utils import run_bass_kernel_spmd

    in_maps, lc2_bias_zero = _prep_in_maps(
        inputs, conv_w, conv_b, lc1_w, lc1_b, lc2_w, lc2_b, d1_w, d1_b, d2_w)
    nc = _get_program(NT, lc2_bias_zero)
    res = run_bass_kernel_spmd(nc, in_maps, list(range(N_CORES)))
    out = np.concatenate([res.results[c]["y"] for c in range(N_CORES)])
    return out.reshape(B_FULL, 1).astype(np.float32)
